# revision 23
# baseline (speedup 1.0000x reference)
"""Fused transformer encoder layer (attention w/ 2D-ALiBi bias + FFN) on 8 trn2 cores.

Sharding: core c handles batch b = c//2, token half h = c%2 (512 query rows).
K/V are computed per-core for the full 1024-token sequence of its batch
(duplicated across the 2 cores sharing a batch); outputs are disjoint row
slices of the final tensor, so no collectives are needed.

Bias trick: the alibi_2d bias slope_h*(|xi-xj|+|yi-yj|) is folded into the
QK^T contraction. |xi-xj| = xi + xj - 2*a_i.a_j with a_i in {0,1}^31 the
threshold indicators of xi, so dist(i,j) = s_i + s_j - 2*c_i.c_j (c = 62-dim
indicator, s = x+y). The per-query term slope*s_i is constant along the
softmax axis and is dropped. Q/K are augmented with 64 extra contraction dims
(s_j / pad / c_j on the K side; 1 / 0 / -2*c_i on the Q side), making the
score contraction K = 64+64 = 128 exactly — full PE array, bias for free.

bf16 precision care: the aug rows are small integers / {0,-2} — exact in
bf16. The attention scale AND the per-head slope are folded out of the bf16
data: Q-projection weights carry scale/slope_h per head (so scores come out
as S/slope_h) and the exact fp32 slope_h is re-applied as the exp()
activation's scale immediate. exp needs no max-subtraction (|S| <= ~50 by
construction).

Scores are computed keys-on-partitions (S^T layout) so the exp() output is
already P^T for the AV matmul (no transpose). Softmax denominators come from
an appended ones-column in V; normalization is deferred past the (linear)
output projection boundary: each head's O^T rows are scaled by a broadcasted
1/den (built with a small fp32 selector matmul) before the head-summing
projection.
"""

import math
import sys
import time

for _p in ("/opt/trn_rl_repo",):
    if _p not in sys.path:
        sys.path.insert(0, _p)

import numpy as np
import ml_dtypes

import concourse.bass as bass
import concourse.tile as tile
from concourse import bacc, mybir
from concourse.masks import make_identity

F32 = mybir.dt.float32
BF16 = mybir.dt.bfloat16
BF = ml_dtypes.bfloat16

D = 1024          # d_model
H = 16            # heads
HD = 64           # head dim
DFF = 4096
B = 4
N = 1024          # sequence length
NT = 512          # tokens (query rows) per core
GRID = 32
EPS = 1e-5
NCORES = 8
SCALE = HD ** -0.5


def _alibi_slopes(n):
    def pow2(n_):
        start = 2.0 ** (-(2.0 ** -(math.log2(n_) - 3)))
        return [start * start ** i for i in range(n_)]
    if math.log2(n).is_integer():
        return np.array(pow2(n), dtype=np.float64)
    m = 2 ** math.floor(math.log2(n))
    s = pow2(m)
    s += [s[-1] * 0.5 ** (i + 1) for i in range(n - m)]
    return np.array(s, dtype=np.float64)


SLOPES = _alibi_slopes(H)


def build_nc(trivial_affine=False):
    """trivial_affine: g1/g2 all-ones and be1/be2/b2 all-zeros -> skip those ops."""
    nc = bacc.Bacc()

    srcT = nc.declare_dram_parameter("srcT", [D, N], BF16, isOutput=False)
    srcQT = nc.declare_dram_parameter("srcQT", [D, NT], BF16, isOutput=False)
    src_rows = nc.declare_dram_parameter("src_rows", [NT, D], F32, isOutput=False)
    WqTs = nc.declare_dram_parameter("WqTs", [D, D], BF16, isOutput=False)
    WkT = nc.declare_dram_parameter("WkT", [D, D], BF16, isOutput=False)
    WvT = nc.declare_dram_parameter("WvT", [D, D], BF16, isOutput=False)
    WoT = nc.declare_dram_parameter("WoT", [D, D], BF16, isOutput=False)
    # W1 pre-swizzled on host: W1S[ft, p, dc*128+j] = W1.T[dc*128+p, ft*128+j]
    # so each FFN1 weight block is one fully-contiguous 256KB DMA.
    W1S = nc.declare_dram_parameter("W1S", [32, 128, D], BF16, isOutput=False)
    W2Tb = nc.declare_dram_parameter("W2Tb", [DFF, D], BF16, isOutput=False)
    kaug_x = nc.declare_dram_parameter("kaug_x", [64, N], BF16, isOutput=False)
    qaug_x = nc.declare_dram_parameter("qaug_x", [H, 64, NT], BF16, isOutput=False)
    selm = nc.declare_dram_parameter("selm", [2, 128], BF16, isOutput=False)
    b1r = nc.declare_dram_parameter("b1r", [128, 32], F32, isOutput=False)
    b2 = nc.declare_dram_parameter("b2", [1, D], F32, isOutput=False)
    g1 = nc.declare_dram_parameter("g1", [1, D], F32, isOutput=False)
    be1 = nc.declare_dram_parameter("be1", [1, D], F32, isOutput=False)
    g2 = nc.declare_dram_parameter("g2", [1, D], F32, isOutput=False)
    be2 = nc.declare_dram_parameter("be2", [1, D], F32, isOutput=False)
    out = nc.declare_dram_parameter("out", [NT, D], F32, isOutput=True)

    AF = mybir.ActivationFunctionType
    OP = mybir.AluOpType

    with tile.TileContext(nc) as tc:
        with (
            tc.tile_pool(name="misc", bufs=1) as misc,
            tc.tile_pool(name="lnp", bufs=4) as lnp,
        ):
            eps_sb = misc.tile([128, 1], F32, tag="eps")
            nc.vector.memset(eps_sb, EPS)
            ident = misc.tile([128, 128], F32, tag="ident")
            make_identity(nc, ident)
            # OT_sb[p, c, q]: head 2c in partitions 0:64, head 2c+1 in 64:128
            OT_sb = misc.tile([128, 8, NT], BF16, tag="otsb")

            def ln_apply(x_ap, gbc, bbc):
                stats = lnp.tile([128, 2, 6], F32, tag="lnstats", name="lnstats")
                for sg in range(2):
                    nc.vector.bn_stats(
                        out=stats[:, sg, :], in_=x_ap[:, sg * 512 : sg * 512 + 512]
                    )
                mv = lnp.tile([128, 2], F32, tag="lnmv", name="lnmv")
                nc.vector.bn_aggr(out=mv, in_=stats)
                nc.scalar.activation(
                    out=mv[:, 1:2], in_=mv[:, 1:2], func=AF.Sqrt,
                    bias=eps_sb, scale=1.0,
                )
                nc.vector.reciprocal(out=mv[:, 1:2], in_=mv[:, 1:2])
                nc.vector.tensor_scalar(
                    out=x_ap, in0=x_ap,
                    scalar1=mv[:, 0:1], scalar2=mv[:, 1:2],
                    op0=OP.subtract, op1=OP.mult,
                )
                if gbc is not None:
                    nc.vector.tensor_mul(out=x_ap, in0=x_ap, in1=gbc)
                if bbc is not None:
                    nc.vector.tensor_add(out=x_ap, in0=x_ap, in1=bbc)

            # ============ attention scope ============
            with tc.tile_pool(name="att", bufs=1) as att:
                kaug = att.tile([128, H, N], BF16, tag="kaug")
                qaug = att.tile([128, H, NT], BF16, tag="qaug")
                v_sb = att.tile([128, 8, H * 65], BF16, tag="vsb")
                v4 = v_sb.rearrange("p m (h w) -> p m h w", w=65)
                nc.vector.memset(v4[:, :, :, 64], 1.0)

                # --- phase 1: projections (all weights resident, bf16) ---
                # DMA emission order tracks first-use order so the PE can
                # start as soon as the Q operands land.
                with tc.tile_pool(name="ph1", bufs=1) as ph1:
                    # fine-grained loads so the first Q matmuls start early
                    sqt = ph1.tile([128, 8, NT], BF16, tag="sqt")
                    sq_vw = srcQT[:, :].rearrange("(c p) n -> p c n", p=128)
                    wqf = ph1.tile([128, 8, D], BF16, tag="wqf")
                    wq_vw = WqTs[:, :].rearrange("(c p) n -> p c n", p=128)
                    for c0 in range(0, 8, 2):
                        nc.sync.dma_start(
                            out=sqt[:, c0 : c0 + 2, :], in_=sq_vw[:, c0 : c0 + 2, :]
                        )
                        nc.sync.dma_start(
                            out=wqf[:, c0 : c0 + 2, :], in_=wq_vw[:, c0 : c0 + 2, :]
                        )
                    stf = ph1.tile([128, 8, N], BF16, tag="stf")
                    st_vw = srcT[:, :].rearrange("(c p) n -> p c n", p=128)
                    wkf = ph1.tile([128, 8, D], BF16, tag="wkf")
                    wk_vw = WkT[:, :].rearrange("(c p) n -> p c n", p=128)
                    for c0 in range(0, 8, 4):
                        nc.sync.dma_start(
                            out=stf[:, c0 : c0 + 4, :], in_=st_vw[:, c0 : c0 + 4, :]
                        )
                        nc.sync.dma_start(
                            out=wkf[:, c0 : c0 + 4, :], in_=wk_vw[:, c0 : c0 + 4, :]
                        )
                    wvf = ph1.tile([128, 8, D], BF16, tag="wvf")
                    nc.sync.dma_start(
                        out=wvf, in_=WvT[:, :].rearrange("(c p) n -> p c n", p=128)
                    )
                    for h in range(H):
                        nc.sync.dma_start(out=kaug[64:128, h, :], in_=kaug_x[:, :])
                        nc.sync.dma_start(out=qaug[64:128, h, :], in_=qaug_x[h, :, :])

                    # Q + K projections -> qaug/kaug top halves (shared pool)
                    with tc.tile_pool(name="psQK", bufs=3, space="PSUM") as psQK:
                        for dt in range(8):
                            qps = psQK.tile([128, NT], F32, tag="proj", name="qps")
                            for dc in range(8):
                                nc.tensor.matmul(
                                    qps,
                                    wqf[:, dc, dt * 128 : dt * 128 + 128],
                                    sqt[:, dc, :],
                                    start=(dc == 0), stop=(dc == 7),
                                )
                            nc.vector.tensor_copy(out=qaug[0:64, 2 * dt, :], in_=qps[0:64, :])
                            nc.vector.tensor_copy(
                                out=qaug[0:64, 2 * dt + 1, :], in_=qps[64:128, :]
                            )
                        for dt in range(8):
                            for mh in range(2):
                                kps = psQK.tile([128, 512], F32, tag="proj", name="kps")
                                for dc in range(8):
                                    nc.tensor.matmul(
                                        kps,
                                        wkf[:, dc, dt * 128 : dt * 128 + 128],
                                        stf[:, dc, mh * 512 : mh * 512 + 512],
                                        start=(dc == 0), stop=(dc == 7),
                                    )
                                nc.vector.tensor_copy(
                                    out=kaug[0:64, 2 * dt, mh * 512 : mh * 512 + 512],
                                    in_=kps[0:64, :],
                                )
                                nc.vector.tensor_copy(
                                    out=kaug[0:64, 2 * dt + 1, mh * 512 : mh * 512 + 512],
                                    in_=kps[64:128, :],
                                )

                    # V projection (natural layout, + ones col); 4-bank groups
                    # so the last PSUM banks free quickly before attention
                    with tc.tile_pool(name="psV", bufs=1, space="PSUM") as psV:
                        for dh in range(2):
                            for mg in range(2):
                                vps = [
                                    psV.tile([128, 512], F32, tag=f"vps{m}", name=f"vps{m}")
                                    for m in range(4)
                                ]
                                for dc in range(8):
                                    for lm in range(4):
                                        mt = mg * 4 + lm
                                        nc.tensor.matmul(
                                            vps[lm],
                                            stf[:, dc, mt * 128 : mt * 128 + 128],
                                            wvf[:, dc, dh * 512 : dh * 512 + 512],
                                            start=(dc == 0), stop=(dc == 7),
                                        )
                                for lm in range(4):
                                    nc.vector.tensor_copy(
                                        out=v4[:, mg * 4 + lm, dh * 8 : dh * 8 + 8, 0:64],
                                        in_=vps[lm].rearrange("p (h w) -> p h w", w=64),
                                    )

                # --- phase 2: attention, 3-stage pipeline:
                #     QKA(h) | QKB(h-1) | AV(h-2) (+ per-pair normalize) ---
                sel_sb = att.tile([2, 128], BF16, tag="sel")
                nc.sync.dma_start(out=sel_sb, in_=selm[:, :])
                with (
                    tc.tile_pool(name="ptp", bufs=3) as ptp,
                    tc.tile_pool(name="stgp", bufs=2) as stgp,
                    tc.tile_pool(name="psST", bufs=1, space="PSUM") as psST,
                    tc.tile_pool(name="psOT", bufs=1, space="PSUM") as psOT,
                    tc.tile_pool(name="psD", bufs=1, space="PSUM") as psD,
                ):
                    pts = {}
                    dpairs = {}
                    for step in range(H + 2):
                        if step < H:
                            # stage 1: scores mt 0-3 + wide exp (amortizes the
                            # ~344-cycle ACT init overhead)
                            h = step
                            pt = ptp.tile([128, 8, NT], BF16, tag="pt", name="pt")
                            pts[h] = pt
                            stA = psST.tile(
                                [128, 4, NT], F32, tag="stA", name="stA", bufs=1
                            )
                            for mt in range(4):
                                nc.tensor.matmul(
                                    stA[:, mt, :],
                                    kaug[:, h, mt * 128 : mt * 128 + 128],
                                    qaug[:, h, :],
                                    start=True, stop=True,
                                )
                            nc.scalar.activation(
                                out=pt[:, 0:4, :], in_=stA, func=AF.Exp,
                                scale=float(SLOPES[h]),
                            )
                        if 1 <= step <= H:
                            # stage 2: scores mt 4-7 for head step-1
                            h = step - 1
                            pt = pts[h]
                            for g in range(2):
                                stB = psST.tile(
                                    [128, 2, NT], F32, tag="stB", name="stB", bufs=1
                                )
                                for j in range(2):
                                    mt = 4 + g * 2 + j
                                    nc.tensor.matmul(
                                        stB[:, j, :],
                                        kaug[:, h, mt * 128 : mt * 128 + 128],
                                        qaug[:, h, :],
                                        start=True, stop=True,
                                    )
                                nc.scalar.activation(
                                    out=pt[:, 4 + g * 2 : 6 + g * 2, :], in_=stB,
                                    func=AF.Exp, scale=float(SLOPES[h]),
                                )
                        if step >= 2:
                            # stage 3: AV for head step-2, + denominator row
                            hp = step - 2
                            ptc = pts.pop(hp)
                            otp = psOT.tile([65, NT], F32, tag="ot", name="otp")
                            for mt in range(8):
                                nc.tensor.matmul(
                                    otp,
                                    v_sb[:, mt, hp * 65 : hp * 65 + 65],
                                    ptc[:, mt, :],
                                    start=(mt == 0), stop=(mt == 7),
                                )
                            base = (hp % 2) * 64
                            nc.vector.tensor_copy(
                                out=OT_sb[base : base + 64, hp // 2, :], in_=otp[0:64, :]
                            )
                            stg = stgp.tile([128, NT], F32, tag="stg", name="stg")
                            nc.vector.tensor_copy(out=stg[64:65, :], in_=otp[64:65, :])
                            if hp % 2 == 0:
                                dpair = stgp.tile(
                                    [2, NT], F32, tag="dpair", name="dpair", bufs=2
                                )
                                dpairs[hp // 2] = dpair
                            dpair = dpairs[hp // 2]
                            nc.sync.dma_start(
                                out=dpair[hp % 2 : hp % 2 + 1, :], in_=stg[64:65, :]
                            )
                            if hp % 2 == 1:
                                # normalize pair c inline: 1/den as bf16 hi+lo,
                                # selector-matmul broadcast, OT *= 1/den
                                c = hp // 2
                                dpi = stgp.tile([2, NT], F32, tag="dpi", name="dpi", bufs=2)
                                nc.vector.reciprocal(out=dpi, in_=dpair)
                                dhi = stgp.tile([2, NT], BF16, tag="dhi", name="dhi", bufs=2)
                                dlo = stgp.tile([2, NT], BF16, tag="dlo", name="dlo", bufs=2)
                                nc.vector.tensor_copy(out=dhi, in_=dpi)
                                nc.vector.tensor_sub(out=dlo, in0=dpi, in1=dhi)
                                dbc = psD.tile([128, NT], F32, tag="dbc", name="dbc")
                                nc.tensor.matmul(dbc, sel_sb, dhi, start=True, stop=False)
                                nc.tensor.matmul(dbc, sel_sb, dlo, start=False, stop=True)
                                nc.vector.tensor_mul(
                                    out=OT_sb[:, c, :], in0=OT_sb[:, c, :], in1=dbc
                                )

            # ============ post-attention scope ============
            with tc.tile_pool(name="ffn", bufs=1) as ffn:
                W2_sb = ffn.tile([128, 32, D], BF16, tag="w2")
                w2_v = W2Tb[:, :].rearrange("(c p) n -> p c n", p=128)
                b1_sb = ffn.tile([128, 32], F32, tag="b1")
                nc.sync.dma_start(out=b1_sb, in_=b1r[:, :])

                x1_sb = ffn.tile([128, 4, D], F32, tag="x1")
                x1T_sb = ffn.tile([128, 8, NT], BF16, tag="x1T")

                # --- phase 3: denominators, out-proj, LN1, transpose ---
                with (
                    tc.tile_pool(name="p3", bufs=1) as p3,
                    tc.tile_pool(name="psD", bufs=2, space="PSUM") as psD,
                    tc.tile_pool(name="psS2", bufs=2, space="PSUM") as psS2,
                    tc.tile_pool(name="psT", bufs=2, space="PSUM") as psT,
                ):
                    if trivial_affine:
                        g1bc = be1bc = None
                    else:
                        g1bc = p3.tile([128, D], F32, tag="g1bc")
                        be1bc = p3.tile([128, D], F32, tag="be1bc")
                        for t_, src_ in ((g1bc, g1), (be1bc, be1)):
                            nc.sync.dma_start(
                                out=t_, in_=src_[:, :].to_broadcast([128, D])
                            )
                    srar = p3.tile([128, 4, D], F32, tag="srcrows")
                    nc.sync.dma_start(
                        out=srar,
                        in_=src_rows[:, :].rearrange("(nt p) d -> p nt d", p=128),
                    )
                    wof = p3.tile([128, 8, D], BF16, tag="wof")
                    nc.sync.dma_start(
                        out=wof, in_=WoT[:, :].rearrange("(c p) n -> p c n", p=128)
                    )

                    def transposes(nt):
                        for c in range(8):
                            tp = psT.tile([128, 128], F32, tag="tp", name="tp")
                            nc.tensor.transpose(
                                tp, x1_sb[:, nt, c * 128 : c * 128 + 128], ident
                            )
                            dst = x1T_sb[:, c, nt * 128 : nt * 128 + 128]
                            if c % 2 == 0:
                                nc.scalar.activation(out=dst, in_=tp, func=AF.Copy)
                            else:
                                nc.vector.tensor_copy(out=dst, in_=tp)

                    for nt in range(4):
                        for dh in range(2):
                            s2 = psS2.tile([128, 512], F32, tag="s2", name="s2")
                            for c in range(8):
                                nc.tensor.matmul(
                                    s2,
                                    OT_sb[:, c, nt * 128 : nt * 128 + 128],
                                    wof[:, c, dh * 512 : dh * 512 + 512],
                                    start=(c == 0), stop=(c == 7),
                                )
                            nc.vector.tensor_add(
                                out=x1_sb[:, nt, dh * 512 : dh * 512 + 512],
                                in0=s2,
                                in1=srar[:, nt, dh * 512 : dh * 512 + 512],
                            )
                        if nt >= 1:
                            transposes(nt - 1)
                        ln_apply(x1_sb[:, nt, :], g1bc, be1bc)
                    transposes(3)

                # --- phase 4: FFN1 (gelu into bf16 h1T) ---
                h1T_sb = ffn.tile([128, 32, NT], BF16, tag="h1T")
                with (
                    tc.tile_pool(name="w1p", bufs=3) as w1p,
                    tc.tile_pool(name="psH", bufs=3, space="PSUM") as psH,
                ):
                    for ft in range(32):
                        w1 = w1p.tile([128, 8, 128], BF16, tag="w1col", name="w1")
                        nc.sync.dma_start(
                            out=w1.rearrange("p c n -> p (c n)"), in_=W1S[ft, :, :]
                        )
                        if ft % 4 == 0:
                            q = ft // 4
                            nc.sync.dma_start(
                                out=W2_sb[:, q * 4 : q * 4 + 4, :],
                                in_=w2_v[:, q * 4 : q * 4 + 4, :],
                            )
                        hps = psH.tile([128, NT], F32, tag="h1", name="hps")
                        for dc in range(8):
                            nc.tensor.matmul(
                                hps, w1[:, dc, :], x1T_sb[:, dc, :],
                                start=(dc == 0), stop=(dc == 7),
                            )
                        nc.scalar.activation(
                            out=h1T_sb[:, ft, :], in_=hps, func=AF.Gelu,
                            bias=b1_sb[:, ft : ft + 1], scale=1.0,
                        )

                # --- phase 5: FFN2 + residual + LN2 + store ---
                out_v = out[:, :].rearrange("(nt p) d -> p nt d", p=128)
                with tc.tile_pool(name="psY", bufs=3, space="PSUM") as psY:
                    if trivial_affine:
                        b2bc = g2bc = be2bc = None
                    else:
                        b2bc = ffn.tile([128, D], F32, tag="b2bc")
                        g2bc = ffn.tile([128, D], F32, tag="g2bc")
                        be2bc = ffn.tile([128, D], F32, tag="be2bc")
                        for t_, src_ in ((b2bc, b2), (g2bc, g2), (be2bc, be2)):
                            nc.sync.dma_start(
                                out=t_, in_=src_[:, :].to_broadcast([128, D])
                            )
                    for nt in range(4):
                        for dh in range(2):
                            yps = psY.tile([128, 512], F32, tag="y", name="yps")
                            for fc in range(32):
                                nc.tensor.matmul(
                                    yps,
                                    h1T_sb[:, fc, nt * 128 : nt * 128 + 128],
                                    W2_sb[:, fc, dh * 512 : dh * 512 + 512],
                                    start=(fc == 0), stop=(fc == 31),
                                )
                            nc.vector.tensor_add(
                                out=x1_sb[:, nt, dh * 512 : dh * 512 + 512],
                                in0=yps,
                                in1=x1_sb[:, nt, dh * 512 : dh * 512 + 512],
                            )
                        if b2bc is not None:
                            nc.vector.tensor_add(
                                out=x1_sb[:, nt, :], in0=x1_sb[:, nt, :], in1=b2bc
                            )
                        ln_apply(x1_sb[:, nt, :], g2bc, be2bc)
                        nc.sync.dma_start(out=out_v[:, nt, :], in_=x1_sb[:, nt, :])

    nc.finalize()
    return nc


def host_prep(inputs):
    """Build the 8 per-core input maps from the full problem inputs."""
    src = np.asarray(inputs["src"], np.float32)
    coords = np.asarray(inputs["coords"])
    Wq = np.asarray(inputs["Wq"], np.float32)
    Wk = np.asarray(inputs["Wk"], np.float32)
    Wv = np.asarray(inputs["Wv"], np.float32)
    Wo = np.asarray(inputs["Wo"], np.float32)
    W1 = np.asarray(inputs["W1"], np.float32)
    b1 = np.asarray(inputs["b1"], np.float32)
    W2 = np.asarray(inputs["W2"], np.float32)
    b2 = np.asarray(inputs["b2"], np.float32)
    g1 = np.asarray(inputs["g1"], np.float32)
    be1 = np.asarray(inputs["be1"], np.float32)
    g2 = np.asarray(inputs["g2"], np.float32)
    be2 = np.asarray(inputs["be2"], np.float32)

    # per-head q scaling: scores are computed as S/slope_h (slope re-applied
    # as the exp scale), so Wq columns of head h carry SCALE/slope_h.
    colscale = (SCALE / SLOPES)[np.repeat(np.arange(H), HD)]  # [D]
    WqTs = (Wq.T * colscale[None, :]).astype(BF)

    shared = {
        "WqTs": WqTs,
        "WkT": np.ascontiguousarray(Wk.T).astype(BF),
        "WvT": np.ascontiguousarray(Wv.T).astype(BF),
        "WoT": np.ascontiguousarray(Wo.T).astype(BF),
        # W1S[ft, p, dc*128+j] = W1.T[dc*128+p, ft*128+j]
        "W1S": np.ascontiguousarray(
            W1.T.reshape(8, 128, 32, 128).transpose(2, 1, 0, 3).reshape(32, 128, D)
        ).astype(BF),
        "W2Tb": np.ascontiguousarray(W2.T).astype(BF),
        "selm": np.repeat(np.eye(2, dtype=np.float32), 64, axis=1).astype(BF),
        "b1r": np.ascontiguousarray(b1.reshape(32, 128).T),
        "b2": b2.reshape(1, D),
        "g1": g1.reshape(1, D),
        "be1": be1.reshape(1, D),
        "g2": g2.reshape(1, D),
        "be2": be2.reshape(1, D),
    }

    in_maps = []
    for c in range(NCORES):
        b = c // 2
        half = c % 2
        rows = slice(half * NT, (half + 1) * NT)
        x = coords[b, :, 0].astype(np.float64)
        y = coords[b, :, 1].astype(np.float64)
        s = (x + y).astype(np.float32)
        thr = np.arange(1, GRID, dtype=np.float64)
        cx = (x[None, :] >= thr[:, None]).astype(np.float32)
        cy = (y[None, :] >= thr[:, None]).astype(np.float32)
        kaug = np.concatenate(
            [s.reshape(1, N), np.zeros((1, N), np.float32), cx, cy], axis=0
        ).astype(BF)
        qaug = np.empty((H, 64, NT), np.float32)
        qaug[:, 0, :] = 1.0
        qaug[:, 1, :] = 0.0
        qaug[:, 2:33, :] = -2.0 * cx[None, :, rows]
        qaug[:, 33:64, :] = -2.0 * cy[None, :, rows]
        srcTb = np.ascontiguousarray(src[b].T)
        m = dict(shared)
        m.update(
            {
                "srcT": srcTb.astype(BF),
                "srcQT": np.ascontiguousarray(srcTb[:, rows]).astype(BF),
                "src_rows": np.ascontiguousarray(src[b, rows, :]),
                "kaug_x": kaug,
                "qaug_x": qaug.astype(BF),
            }
        )
        in_maps.append(m)
    return in_maps


_NCS = {}
LAST_RUN_S = None


def get_nc(trivial_affine=True):
    if trivial_affine not in _NCS:
        _NCS[trivial_affine] = build_nc(trivial_affine)
    return _NCS[trivial_affine]


def _affine_trivial(inputs):
    return (
        np.all(np.asarray(inputs["g1"]) == 1.0)
        and np.all(np.asarray(inputs["g2"]) == 1.0)
        and not np.any(np.asarray(inputs["be1"]))
        and not np.any(np.asarray(inputs["be2"]))
        and not np.any(np.asarray(inputs["b2"]))
    )


def kernel(**inputs):
    global LAST_RUN_S
    from concourse.bass_utils import run_bass_kernel_spmd

    nc = get_nc(bool(_affine_trivial(inputs)))
    in_maps = host_prep(inputs)
    t0 = time.monotonic()
    res = run_bass_kernel_spmd(nc, in_maps, list(range(NCORES)))
    LAST_RUN_S = time.monotonic() - t0
    full = np.empty((B, N, D), np.float32)
    for c in range(NCORES):
        b = c // 2
        half = c % 2
        full[b, half * NT : (half + 1) * NT, :] = res.results[c]["out"]
    return full


# revision 29
# speedup vs baseline: 15419.0941x; 15419.0941x over previous
"""Fused transformer encoder layer (attention w/ 2D-ALiBi bias + FFN) on 8 trn2 cores.

Sharding: core c handles batch b = c//2, token half h = c%2 (512 query rows).
K/V are computed per-core for the full 1024-token sequence of its batch
(duplicated across the 2 cores sharing a batch); outputs are disjoint row
slices of the final tensor, so no collectives are needed.

Bias trick: the alibi_2d bias slope_h*(|xi-xj|+|yi-yj|) is folded into the
QK^T contraction. |xi-xj| = xi + xj - 2*a_i.a_j with a_i in {0,1}^31 the
threshold indicators of xi, so dist(i,j) = s_i + s_j - 2*c_i.c_j (c = 62-dim
indicator, s = x+y). The per-query term slope*s_i is constant along the
softmax axis and is dropped. Q/K are augmented with 64 extra contraction dims
(s_j / pad / c_j on the K side; 1 / 0 / -2*c_i on the Q side), making the
score contraction K = 64+64 = 128 exactly — full PE array, bias for free.

bf16 precision care: the aug rows are small integers / {0,-2} — exact in
bf16. The attention scale AND the per-head slope are folded out of the bf16
data: Q-projection weights carry scale/slope_h per head (so scores come out
as S/slope_h) and the exact fp32 slope_h is re-applied as the exp()
activation's scale immediate. exp needs no max-subtraction (|S| <= ~50 by
construction).

Scores are computed keys-on-partitions (S^T layout) so the exp() output is
already P^T for the AV matmul (no transpose). Softmax denominators come from
an appended ones-column in V; normalization is deferred past the (linear)
output projection boundary: each head's O^T rows are scaled by a broadcasted
1/den (built with a small fp32 selector matmul) before the head-summing
projection.
"""

import math
import sys
import time

for _p in ("/opt/trn_rl_repo",):
    if _p not in sys.path:
        sys.path.insert(0, _p)

import numpy as np
import ml_dtypes

import concourse.bass as bass
import concourse.tile as tile
from concourse import bacc, mybir
from concourse.masks import make_identity

F32 = mybir.dt.float32
BF16 = mybir.dt.bfloat16
BF = ml_dtypes.bfloat16

D = 1024          # d_model
H = 16            # heads
HD = 64           # head dim
DFF = 4096
B = 4
N = 1024          # sequence length
NT = 512          # tokens (query rows) per core
GRID = 32
EPS = 1e-5
NCORES = 8
SCALE = HD ** -0.5


def _alibi_slopes(n):
    def pow2(n_):
        start = 2.0 ** (-(2.0 ** -(math.log2(n_) - 3)))
        return [start * start ** i for i in range(n_)]
    if math.log2(n).is_integer():
        return np.array(pow2(n), dtype=np.float64)
    m = 2 ** math.floor(math.log2(n))
    s = pow2(m)
    s += [s[-1] * 0.5 ** (i + 1) for i in range(n - m)]
    return np.array(s, dtype=np.float64)


SLOPES = _alibi_slopes(H)


def build_nc(trivial_affine=False):
    """trivial_affine: g1/g2 all-ones and be1/be2/b2 all-zeros -> skip those ops."""
    nc = bacc.Bacc()

    srcT = nc.declare_dram_parameter("srcT", [D, N], BF16, isOutput=False)
    srcQT = nc.declare_dram_parameter("srcQT", [D, NT], BF16, isOutput=False)
    src_rows = nc.declare_dram_parameter("src_rows", [NT, D], F32, isOutput=False)
    WqTs = nc.declare_dram_parameter("WqTs", [D, D], BF16, isOutput=False)
    WkT = nc.declare_dram_parameter("WkT", [D, D], BF16, isOutput=False)
    WvT = nc.declare_dram_parameter("WvT", [D, D], BF16, isOutput=False)
    WoT = nc.declare_dram_parameter("WoT", [D, D], BF16, isOutput=False)
    # W1 pre-swizzled on host: W1S[ft, p, dc*128+j] = W1.T[dc*128+p, ft*128+j]
    # so each FFN1 weight block is one fully-contiguous 256KB DMA.
    W1S = nc.declare_dram_parameter("W1S", [32, 128, D], BF16, isOutput=False)
    W2Tb = nc.declare_dram_parameter("W2Tb", [DFF, D], BF16, isOutput=False)
    kaug_x = nc.declare_dram_parameter("kaug_x", [64, N], BF16, isOutput=False)
    qaug_x = nc.declare_dram_parameter("qaug_x", [H, 64, NT], BF16, isOutput=False)
    selm = nc.declare_dram_parameter("selm", [2, 128], BF16, isOutput=False)
    b1r = nc.declare_dram_parameter("b1r", [128, 32], F32, isOutput=False)
    b2 = nc.declare_dram_parameter("b2", [1, D], F32, isOutput=False)
    g1 = nc.declare_dram_parameter("g1", [1, D], F32, isOutput=False)
    be1 = nc.declare_dram_parameter("be1", [1, D], F32, isOutput=False)
    g2 = nc.declare_dram_parameter("g2", [1, D], F32, isOutput=False)
    be2 = nc.declare_dram_parameter("be2", [1, D], F32, isOutput=False)
    out = nc.declare_dram_parameter("out", [NT, D], F32, isOutput=True)

    AF = mybir.ActivationFunctionType
    OP = mybir.AluOpType

    with tile.TileContext(nc) as tc:
        with (
            tc.tile_pool(name="misc", bufs=1) as misc,
            tc.tile_pool(name="lnp", bufs=4) as lnp,
        ):
            eps_sb = misc.tile([128, 1], F32, tag="eps")
            nc.vector.memset(eps_sb, EPS)
            ident = misc.tile([128, 128], F32, tag="ident")
            make_identity(nc, ident)
            # OT_sb[p, c, q]: head 2c in partitions 0:64, head 2c+1 in 64:128
            OT_sb = misc.tile([128, 8, NT], BF16, tag="otsb")

            def ln_apply(x_ap, gbc, bbc):
                stats = lnp.tile([128, 2, 6], F32, tag="lnstats", name="lnstats")
                for sg in range(2):
                    nc.vector.bn_stats(
                        out=stats[:, sg, :], in_=x_ap[:, sg * 512 : sg * 512 + 512]
                    )
                mv = lnp.tile([128, 2], F32, tag="lnmv", name="lnmv")
                nc.vector.bn_aggr(out=mv, in_=stats)
                nc.scalar.activation(
                    out=mv[:, 1:2], in_=mv[:, 1:2], func=AF.Sqrt,
                    bias=eps_sb, scale=1.0,
                )
                nc.vector.reciprocal(out=mv[:, 1:2], in_=mv[:, 1:2])
                nc.vector.tensor_scalar(
                    out=x_ap, in0=x_ap,
                    scalar1=mv[:, 0:1], scalar2=mv[:, 1:2],
                    op0=OP.subtract, op1=OP.mult,
                )
                if gbc is not None:
                    nc.vector.tensor_mul(out=x_ap, in0=x_ap, in1=gbc)
                if bbc is not None:
                    nc.vector.tensor_add(out=x_ap, in0=x_ap, in1=bbc)

            # ============ attention scope ============
            with tc.tile_pool(name="att", bufs=1) as att:
                kaug = att.tile([128, H, N], BF16, tag="kaug")
                qaug = att.tile([128, H, NT], BF16, tag="qaug")
                v_sb = att.tile([128, 8, H * 65], BF16, tag="vsb")
                v4 = v_sb.rearrange("p m (h w) -> p m h w", w=65)
                nc.vector.memset(v4[:, :, :, 64], 1.0)

                # --- phase 1: projections (all weights resident, bf16) ---
                # DMA emission order tracks first-use order so the PE can
                # start as soon as the Q operands land.
                with tc.tile_pool(name="ph1", bufs=1) as ph1:
                    # fine-grained loads so the first Q matmuls start early
                    sqt = ph1.tile([128, 8, NT], BF16, tag="sqt")
                    sq_vw = srcQT[:, :].rearrange("(c p) n -> p c n", p=128)
                    wqf = ph1.tile([128, 8, D], BF16, tag="wqf")
                    wq_vw = WqTs[:, :].rearrange("(c p) n -> p c n", p=128)
                    for c0, cn in ((0, 1), (1, 1), (2, 2), (4, 2), (6, 2)):
                        nc.sync.dma_start(
                            out=sqt[:, c0 : c0 + cn, :], in_=sq_vw[:, c0 : c0 + cn, :]
                        )
                        nc.sync.dma_start(
                            out=wqf[:, c0 : c0 + cn, :], in_=wq_vw[:, c0 : c0 + cn, :]
                        )
                    stf = ph1.tile([128, 8, N], BF16, tag="stf")
                    st_vw = srcT[:, :].rearrange("(c p) n -> p c n", p=128)
                    wkf = ph1.tile([128, 8, D], BF16, tag="wkf")
                    wk_vw = WkT[:, :].rearrange("(c p) n -> p c n", p=128)
                    for c0 in range(0, 8, 4):
                        nc.sync.dma_start(
                            out=stf[:, c0 : c0 + 4, :], in_=st_vw[:, c0 : c0 + 4, :]
                        )
                        nc.sync.dma_start(
                            out=wkf[:, c0 : c0 + 4, :], in_=wk_vw[:, c0 : c0 + 4, :]
                        )
                    wvf = ph1.tile([128, 8, D], BF16, tag="wvf")
                    nc.sync.dma_start(
                        out=wvf, in_=WvT[:, :].rearrange("(c p) n -> p c n", p=128)
                    )
                    for h in range(H):
                        nc.sync.dma_start(out=kaug[64:128, h, :], in_=kaug_x[:, :])
                        nc.sync.dma_start(out=qaug[64:128, h, :], in_=qaug_x[h, :, :])

                    # Q + K projections -> qaug/kaug top halves (shared pool)
                    with tc.tile_pool(name="psQK", bufs=3, space="PSUM") as psQK:
                        for dt in range(8):
                            qps = psQK.tile([128, NT], F32, tag="proj", name="qps")
                            for dc in range(8):
                                nc.tensor.matmul(
                                    qps,
                                    wqf[:, dc, dt * 128 : dt * 128 + 128],
                                    sqt[:, dc, :],
                                    start=(dc == 0), stop=(dc == 7),
                                )
                            nc.vector.tensor_copy(out=qaug[0:64, 2 * dt, :], in_=qps[0:64, :])
                            nc.vector.tensor_copy(
                                out=qaug[0:64, 2 * dt + 1, :], in_=qps[64:128, :]
                            )
                        for dt in range(8):
                            for mh in range(2):
                                kps = psQK.tile([128, 512], F32, tag="proj", name="kps")
                                for dc in range(8):
                                    nc.tensor.matmul(
                                        kps,
                                        wkf[:, dc, dt * 128 : dt * 128 + 128],
                                        stf[:, dc, mh * 512 : mh * 512 + 512],
                                        start=(dc == 0), stop=(dc == 7),
                                    )
                                nc.vector.tensor_copy(
                                    out=kaug[0:64, 2 * dt, mh * 512 : mh * 512 + 512],
                                    in_=kps[0:64, :],
                                )
                                nc.vector.tensor_copy(
                                    out=kaug[0:64, 2 * dt + 1, mh * 512 : mh * 512 + 512],
                                    in_=kps[64:128, :],
                                )

                    # V projection (natural layout, + ones col); 4-bank groups
                    # so the last PSUM banks free quickly before attention
                    with tc.tile_pool(name="psV", bufs=1, space="PSUM") as psV:
                        for dh in range(2):
                            for mg in range(2):
                                vps = [
                                    psV.tile([128, 512], F32, tag=f"vps{m}", name=f"vps{m}")
                                    for m in range(4)
                                ]
                                for dc in range(8):
                                    for lm in range(4):
                                        mt = mg * 4 + lm
                                        nc.tensor.matmul(
                                            vps[lm],
                                            stf[:, dc, mt * 128 : mt * 128 + 128],
                                            wvf[:, dc, dh * 512 : dh * 512 + 512],
                                            start=(dc == 0), stop=(dc == 7),
                                        )
                                for lm in range(4):
                                    nc.vector.tensor_copy(
                                        out=v4[:, mg * 4 + lm, dh * 8 : dh * 8 + 8, 0:64],
                                        in_=vps[lm].rearrange("p (h w) -> p h w", w=64),
                                    )

                # --- phase 2: attention, 3-stage pipeline:
                #     QKA(h) | QKB(h-1) | AV(h-2) (+ per-pair normalize) ---
                sel_sb = att.tile([2, 128], BF16, tag="sel")
                nc.sync.dma_start(out=sel_sb, in_=selm[:, :])
                with (
                    tc.tile_pool(name="ptp", bufs=3) as ptp,
                    tc.tile_pool(name="stgp", bufs=2) as stgp,
                    tc.tile_pool(name="psST", bufs=1, space="PSUM") as psST,
                    tc.tile_pool(name="psOT", bufs=1, space="PSUM") as psOT,
                    tc.tile_pool(name="psD", bufs=1, space="PSUM") as psD,
                ):
                    pts = {}
                    dpairs = {}
                    for step in range(H + 2):
                        if step < H:
                            # stage 1: scores mt 0-3 + wide exp (amortizes the
                            # ~344-cycle ACT init overhead)
                            h = step
                            pt = ptp.tile([128, 8, NT], BF16, tag="pt", name="pt")
                            pts[h] = pt
                            stA = psST.tile(
                                [128, 4, NT], F32, tag="stA", name="stA", bufs=1
                            )
                            for mt in range(4):
                                nc.tensor.matmul(
                                    stA[:, mt, :],
                                    kaug[:, h, mt * 128 : mt * 128 + 128],
                                    qaug[:, h, :],
                                    start=True, stop=True,
                                )
                            nc.scalar.activation(
                                out=pt[:, 0:4, :], in_=stA, func=AF.Exp,
                                scale=float(SLOPES[h]),
                            )
                        if 1 <= step <= H:
                            # stage 2: scores mt 4-7 for head step-1
                            h = step - 1
                            pt = pts[h]
                            for g in range(2):
                                stB = psST.tile(
                                    [128, 2, NT], F32, tag="stB", name="stB", bufs=1
                                )
                                for j in range(2):
                                    mt = 4 + g * 2 + j
                                    nc.tensor.matmul(
                                        stB[:, j, :],
                                        kaug[:, h, mt * 128 : mt * 128 + 128],
                                        qaug[:, h, :],
                                        start=True, stop=True,
                                    )
                                nc.scalar.activation(
                                    out=pt[:, 4 + g * 2 : 6 + g * 2, :], in_=stB,
                                    func=AF.Exp, scale=float(SLOPES[h]),
                                )
                        if step >= 2:
                            # stage 3: AV for head step-2, + denominator row
                            hp = step - 2
                            ptc = pts.pop(hp)
                            otp = psOT.tile([65, NT], F32, tag="ot", name="otp")
                            for mt in range(8):
                                nc.tensor.matmul(
                                    otp,
                                    v_sb[:, mt, hp * 65 : hp * 65 + 65],
                                    ptc[:, mt, :],
                                    start=(mt == 0), stop=(mt == 7),
                                )
                            base = (hp % 2) * 64
                            nc.vector.tensor_copy(
                                out=OT_sb[base : base + 64, hp // 2, :], in_=otp[0:64, :]
                            )
                            stg = stgp.tile([128, NT], F32, tag="stg", name="stg")
                            nc.vector.tensor_copy(out=stg[64:65, :], in_=otp[64:65, :])
                            if hp % 2 == 0:
                                dpair = stgp.tile(
                                    [2, NT], F32, tag="dpair", name="dpair", bufs=2
                                )
                                dpairs[hp // 2] = dpair
                            dpair = dpairs[hp // 2]
                            nc.sync.dma_start(
                                out=dpair[hp % 2 : hp % 2 + 1, :], in_=stg[64:65, :]
                            )
                            if hp % 2 == 1:
                                # normalize pair c inline: 1/den as bf16 hi+lo,
                                # selector-matmul broadcast, OT *= 1/den
                                c = hp // 2
                                dpi = stgp.tile([2, NT], F32, tag="dpi", name="dpi", bufs=2)
                                nc.vector.reciprocal(out=dpi, in_=dpair)
                                dhi = stgp.tile([2, NT], BF16, tag="dhi", name="dhi", bufs=2)
                                dlo = stgp.tile([2, NT], BF16, tag="dlo", name="dlo", bufs=2)
                                nc.vector.tensor_copy(out=dhi, in_=dpi)
                                nc.vector.tensor_sub(out=dlo, in0=dpi, in1=dhi)
                                dbc = psD.tile([128, NT], F32, tag="dbc", name="dbc")
                                nc.tensor.matmul(dbc, sel_sb, dhi, start=True, stop=False)
                                nc.tensor.matmul(dbc, sel_sb, dlo, start=False, stop=True)
                                nc.vector.tensor_mul(
                                    out=OT_sb[:, c, :], in0=OT_sb[:, c, :], in1=dbc
                                )

            # ============ post-attention scope ============
            with tc.tile_pool(name="ffn", bufs=1) as ffn:
                W2_sb = ffn.tile([128, 32, D], BF16, tag="w2")
                w2_v = W2Tb[:, :].rearrange("(c p) n -> p c n", p=128)
                b1_sb = ffn.tile([128, 32], F32, tag="b1")
                nc.sync.dma_start(out=b1_sb, in_=b1r[:, :])

                x1_sb = ffn.tile([128, 4, D], F32, tag="x1")
                x1T_sb = ffn.tile([128, 8, NT], BF16, tag="x1T")

                # --- phase 3: denominators, out-proj, LN1, transpose ---
                with (
                    tc.tile_pool(name="p3", bufs=1) as p3,
                    tc.tile_pool(name="psD", bufs=2, space="PSUM") as psD,
                    tc.tile_pool(name="psS2", bufs=2, space="PSUM") as psS2,
                    tc.tile_pool(name="psT", bufs=2, space="PSUM") as psT,
                ):
                    if trivial_affine:
                        g1bc = be1bc = None
                    else:
                        g1bc = p3.tile([128, D], F32, tag="g1bc")
                        be1bc = p3.tile([128, D], F32, tag="be1bc")
                        for t_, src_ in ((g1bc, g1), (be1bc, be1)):
                            nc.sync.dma_start(
                                out=t_, in_=src_[:, :].to_broadcast([128, D])
                            )
                    srar = p3.tile([128, 4, D], F32, tag="srcrows")
                    nc.sync.dma_start(
                        out=srar,
                        in_=src_rows[:, :].rearrange("(nt p) d -> p nt d", p=128),
                    )
                    wof = p3.tile([128, 8, D], BF16, tag="wof")
                    nc.sync.dma_start(
                        out=wof, in_=WoT[:, :].rearrange("(c p) n -> p c n", p=128)
                    )

                    def transposes(nt):
                        for c in range(8):
                            tp = psT.tile([128, 128], F32, tag="tp", name="tp")
                            nc.tensor.transpose(
                                tp, x1_sb[:, nt, c * 128 : c * 128 + 128], ident
                            )
                            dst = x1T_sb[:, c, nt * 128 : nt * 128 + 128]
                            if c % 2 == 0:
                                nc.scalar.activation(out=dst, in_=tp, func=AF.Copy)
                            else:
                                nc.vector.tensor_copy(out=dst, in_=tp)

                    for nt in range(4):
                        for dh in range(2):
                            s2 = psS2.tile([128, 512], F32, tag="s2", name="s2")
                            for c in range(8):
                                nc.tensor.matmul(
                                    s2,
                                    OT_sb[:, c, nt * 128 : nt * 128 + 128],
                                    wof[:, c, dh * 512 : dh * 512 + 512],
                                    start=(c == 0), stop=(c == 7),
                                )
                            nc.vector.tensor_add(
                                out=x1_sb[:, nt, dh * 512 : dh * 512 + 512],
                                in0=s2,
                                in1=srar[:, nt, dh * 512 : dh * 512 + 512],
                            )
                        if nt >= 1:
                            transposes(nt - 1)
                        ln_apply(x1_sb[:, nt, :], g1bc, be1bc)
                    transposes(3)

                # --- phase 4: FFN1 (gelu into bf16 h1T) ---
                h1T_sb = ffn.tile([128, 32, NT], BF16, tag="h1T")
                with (
                    tc.tile_pool(name="w1p", bufs=3) as w1p,
                    tc.tile_pool(name="psH", bufs=3, space="PSUM") as psH,
                ):
                    for ft in range(32):
                        w1 = w1p.tile([128, 8, 128], BF16, tag="w1col", name="w1")
                        nc.sync.dma_start(
                            out=w1.rearrange("p c n -> p (c n)"), in_=W1S[ft, :, :]
                        )
                        if ft % 4 == 0:
                            q = ft // 4
                            nc.sync.dma_start(
                                out=W2_sb[:, q * 4 : q * 4 + 4, :],
                                in_=w2_v[:, q * 4 : q * 4 + 4, :],
                            )
                        hps = psH.tile([128, NT], F32, tag="h1", name="hps")
                        for dc in range(8):
                            nc.tensor.matmul(
                                hps, w1[:, dc, :], x1T_sb[:, dc, :],
                                start=(dc == 0), stop=(dc == 7),
                            )
                        nc.scalar.activation(
                            out=h1T_sb[:, ft, :], in_=hps, func=AF.Gelu,
                            bias=b1_sb[:, ft : ft + 1], scale=1.0,
                        )

                # --- phase 5: FFN2 + residual + LN2 + store ---
                out_v = out[:, :].rearrange("(nt p) d -> p nt d", p=128)
                with tc.tile_pool(name="psY", bufs=3, space="PSUM") as psY:
                    if trivial_affine:
                        b2bc = g2bc = be2bc = None
                    else:
                        b2bc = ffn.tile([128, D], F32, tag="b2bc")
                        g2bc = ffn.tile([128, D], F32, tag="g2bc")
                        be2bc = ffn.tile([128, D], F32, tag="be2bc")
                        for t_, src_ in ((b2bc, b2), (g2bc, g2), (be2bc, be2)):
                            nc.sync.dma_start(
                                out=t_, in_=src_[:, :].to_broadcast([128, D])
                            )
                    for nt in range(4):
                        for dh in range(2):
                            yps = psY.tile([128, 512], F32, tag="y", name="yps")
                            for fc in range(32):
                                nc.tensor.matmul(
                                    yps,
                                    h1T_sb[:, fc, nt * 128 : nt * 128 + 128],
                                    W2_sb[:, fc, dh * 512 : dh * 512 + 512],
                                    start=(fc == 0), stop=(fc == 31),
                                )
                            nc.vector.tensor_add(
                                out=x1_sb[:, nt, dh * 512 : dh * 512 + 512],
                                in0=yps,
                                in1=x1_sb[:, nt, dh * 512 : dh * 512 + 512],
                            )
                        if b2bc is not None:
                            nc.vector.tensor_add(
                                out=x1_sb[:, nt, :], in0=x1_sb[:, nt, :], in1=b2bc
                            )
                        ln_apply(x1_sb[:, nt, :], g2bc, be2bc)
                        nc.sync.dma_start(out=out_v[:, nt, :], in_=x1_sb[:, nt, :])

    nc.finalize()
    return nc


def host_prep(inputs):
    """Build the 8 per-core input maps from the full problem inputs."""
    src = np.asarray(inputs["src"], np.float32)
    coords = np.asarray(inputs["coords"])
    Wq = np.asarray(inputs["Wq"], np.float32)
    Wk = np.asarray(inputs["Wk"], np.float32)
    Wv = np.asarray(inputs["Wv"], np.float32)
    Wo = np.asarray(inputs["Wo"], np.float32)
    W1 = np.asarray(inputs["W1"], np.float32)
    b1 = np.asarray(inputs["b1"], np.float32)
    W2 = np.asarray(inputs["W2"], np.float32)
    b2 = np.asarray(inputs["b2"], np.float32)
    g1 = np.asarray(inputs["g1"], np.float32)
    be1 = np.asarray(inputs["be1"], np.float32)
    g2 = np.asarray(inputs["g2"], np.float32)
    be2 = np.asarray(inputs["be2"], np.float32)

    # per-head q scaling: scores are computed as S/slope_h (slope re-applied
    # as the exp scale), so Wq columns of head h carry SCALE/slope_h.
    colscale = (SCALE / SLOPES)[np.repeat(np.arange(H), HD)]  # [D]
    WqTs = (Wq.T * colscale[None, :]).astype(BF)

    shared = {
        "WqTs": WqTs,
        "WkT": np.ascontiguousarray(Wk.T).astype(BF),
        "WvT": np.ascontiguousarray(Wv.T).astype(BF),
        "WoT": np.ascontiguousarray(Wo.T).astype(BF),
        # W1S[ft, p, dc*128+j] = W1.T[dc*128+p, ft*128+j]
        "W1S": np.ascontiguousarray(
            W1.T.reshape(8, 128, 32, 128).transpose(2, 1, 0, 3).reshape(32, 128, D)
        ).astype(BF),
        "W2Tb": np.ascontiguousarray(W2.T).astype(BF),
        "selm": np.repeat(np.eye(2, dtype=np.float32), 64, axis=1).astype(BF),
        "b1r": np.ascontiguousarray(b1.reshape(32, 128).T),
        "b2": b2.reshape(1, D),
        "g1": g1.reshape(1, D),
        "be1": be1.reshape(1, D),
        "g2": g2.reshape(1, D),
        "be2": be2.reshape(1, D),
    }

    in_maps = []
    for c in range(NCORES):
        b = c // 2
        half = c % 2
        rows = slice(half * NT, (half + 1) * NT)
        x = coords[b, :, 0].astype(np.float64)
        y = coords[b, :, 1].astype(np.float64)
        s = (x + y).astype(np.float32)
        thr = np.arange(1, GRID, dtype=np.float64)
        cx = (x[None, :] >= thr[:, None]).astype(np.float32)
        cy = (y[None, :] >= thr[:, None]).astype(np.float32)
        kaug = np.concatenate(
            [s.reshape(1, N), np.zeros((1, N), np.float32), cx, cy], axis=0
        ).astype(BF)
        qaug = np.empty((H, 64, NT), np.float32)
        qaug[:, 0, :] = 1.0
        qaug[:, 1, :] = 0.0
        qaug[:, 2:33, :] = -2.0 * cx[None, :, rows]
        qaug[:, 33:64, :] = -2.0 * cy[None, :, rows]
        srcTb = np.ascontiguousarray(src[b].T)
        m = dict(shared)
        m.update(
            {
                "srcT": srcTb.astype(BF),
                "srcQT": np.ascontiguousarray(srcTb[:, rows]).astype(BF),
                "src_rows": np.ascontiguousarray(src[b, rows, :]),
                "kaug_x": kaug,
                "qaug_x": qaug.astype(BF),
            }
        )
        in_maps.append(m)
    return in_maps


_NCS = {}
LAST_RUN_S = None


def get_nc(trivial_affine=True):
    if trivial_affine not in _NCS:
        _NCS[trivial_affine] = build_nc(trivial_affine)
    return _NCS[trivial_affine]


def _affine_trivial(inputs):
    return (
        np.all(np.asarray(inputs["g1"]) == 1.0)
        and np.all(np.asarray(inputs["g2"]) == 1.0)
        and not np.any(np.asarray(inputs["be1"]))
        and not np.any(np.asarray(inputs["be2"]))
        and not np.any(np.asarray(inputs["b2"]))
    )


def kernel(**inputs):
    global LAST_RUN_S
    from concourse.bass_utils import run_bass_kernel_spmd

    nc = get_nc(bool(_affine_trivial(inputs)))
    in_maps = host_prep(inputs)
    t0 = time.monotonic()
    res = run_bass_kernel_spmd(nc, in_maps, list(range(NCORES)))
    LAST_RUN_S = time.monotonic() - t0
    full = np.empty((B, N, D), np.float32)
    for c in range(NCORES):
        b = c // 2
        half = c % 2
        full[b, half * NT : (half + 1) * NT, :] = res.results[c]["out"]
    return full
